# revision 1
# baseline (speedup 1.0000x reference)
"""CrossFrameAttention Trainium2 kernel.

Computes, per spatial location n (of B*H*W) with C=128 channels, NH=4 heads:
  q = Wq x_t[n];  k_s = Wk x_s[n];  v_s = Wv x_s[n]   (s in {t,p,n})
  score_s[h] = <q_h, k_{s,h}> / sqrt(hd);  a = softmax over s (3-way)
  ctx = sum_s a_s * v_s;  y = Wo ctx + x_t[n];  out = LayerNorm_C(y)*gamma+beta

Sharding: pure data parallel, one batch element per NeuronCore (B=8 = n_cores).
Each core processes its [C, H*W] = [128, 16384] slab in column tiles of T=512.

Layout is channels-on-partitions ([C, T]); per-head/partition-direction
reductions and broadcasts are done with small matmuls (mask / ones / identity
stationary operands). Softmax+LayerNorm scalar lanes are [4, T] / [1, T].
"""

import math
import os
import sys
from contextlib import ExitStack

import numpy as np

sys.path.insert(0, "/opt/trn_rl_repo")

import concourse.bass as bass
import concourse.tile as tile
from concourse import mybir
from concourse.bass_utils import run_bass_kernel_spmd

C = 128
NH = 4
HD = C // NH
B = 8
HW = 128 * 128
T = 512
LN_EPS = 1e-5

F32 = mybir.dt.float32
AF = mybir.ActivationFunctionType
ALU = mybir.AluOpType

# --- toggles (perf experiments) ---
Q_VIA_ACT_COPY = True    # copy Q PSUM->SBUF on ScalarE before P=Q*K muls
V_VIA_ACT_COPY = True    # copy V PSUM->SBUF on ScalarE before G=A*V muls
SPLIT_WAITS = True       # walrus workaround; breaks CoreSim, needed on HW

X_BUFS = 3
SB_BUFS = 2
N_CORES = 8
ABL_DVE = 0


def _split_sync_waits(nc: bass.Bass, max_waits: int = 1) -> None:
    """This walrus build's instruction encoders reject instructions carrying
    more than a couple of semaphore waits; hoist extras onto preceding
    Drains on the same engine (serial on the engine => equivalent)."""
    f = nc.m.functions[0]
    n = 0
    for bb in f.blocks:
        new = []
        for inst in bb.instructions:
            si = getattr(inst, "sync_info", None)
            limit = 1 if isinstance(inst, mybir.InstDrain) else max_waits
            if si is not None and si.on_wait and len(si.on_wait) > limit:
                waits = list(si.on_wait)
                extra, keep = waits[:-limit], waits[-limit:]
                for k in range(0, len(extra), 1):
                    n += 1
                    new.append(mybir.InstDrain(
                        name=f"{inst.name}-sw{k}",
                        engine=inst.engine,
                        ins=[], outs=[],
                        sync_info=mybir.SyncInfo(
                            on_wait=[extra[k]], on_update=[]),
                    ))
                inst.sync_info = mybir.SyncInfo(
                    on_wait=keep, on_update=list(si.on_update))
            new.append(inst)
        bb.instructions[:] = new


def build_nc(ncols: int, has_bias: bool, has_gb: bool) -> bass.Bass:
    nt = ncols // T
    assert ncols % T == 0

    nc = bass.Bass("TRN2")

    x_t = nc.dram_tensor("x_t", [C, ncols], F32, kind="ExternalInput")
    x_p = nc.dram_tensor("x_p", [C, ncols], F32, kind="ExternalInput")
    x_n = nc.dram_tensor("x_n", [C, ncols], F32, kind="ExternalInput")
    out = nc.dram_tensor("out", [C, ncols], F32, kind="ExternalOutput")

    wqT = nc.dram_tensor("wqT", [C, C], F32, kind="ExternalInput")
    wkT = nc.dram_tensor("wkT", [C, C], F32, kind="ExternalInput")
    wvT = nc.dram_tensor("wvT", [C, C], F32, kind="ExternalInput")
    woT = nc.dram_tensor("woT", [C, C], F32, kind="ExternalInput")
    maskT = nc.dram_tensor("maskT", [NH, C], F32, kind="ExternalInput")
    maskC = nc.dram_tensor("maskC", [C, NH], F32, kind="ExternalInput")
    ident = nc.dram_tensor("ident", [C, C], F32, kind="ExternalInput")
    onesC = nc.dram_tensor("onesC", [C, 1], F32, kind="ExternalInput")
    ones1 = nc.dram_tensor("ones1", [1, C], F32, kind="ExternalInput")
    if has_bias:
        bq = nc.dram_tensor("bq", [C, 1], F32, kind="ExternalInput")
        bk = nc.dram_tensor("bk", [C, 1], F32, kind="ExternalInput")
        bv = nc.dram_tensor("bv", [C, 1], F32, kind="ExternalInput")
        bo = nc.dram_tensor("bo", [C, 1], F32, kind="ExternalInput")
    if has_gb:
        gam = nc.dram_tensor("gam", [C, 1], F32, kind="ExternalInput")
        bet = nc.dram_tensor("bet", [C, 1], F32, kind="ExternalInput")

    with tile.TileContext(nc) as tc, ExitStack() as ctx:
        const = ctx.enter_context(tc.tile_pool(name="const", bufs=1))
        xpool = ctx.enter_context(tc.tile_pool(name="xpool", bufs=X_BUFS))
        sb = ctx.enter_context(tc.tile_pool(name="sb", bufs=SB_BUFS))
        ps1 = ctx.enter_context(tc.tile_pool(name="ps1", bufs=1, space="PSUM"))

        def cload(dr, shape):
            t = const.tile(shape, F32, tag=f"c_{dr.name}")
            nc.sync.dma_start(t, dr[:, :])
            return t

        c_wqT = cload(wqT, [C, C])
        c_wkT = cload(wkT, [C, C])
        c_wvT = cload(wvT, [C, C])
        c_woT = cload(woT, [C, C])
        c_maskT = cload(maskT, [NH, C])
        c_maskC = cload(maskC, [C, NH])
        c_ident = cload(ident, [C, C])
        c_onesC = cload(onesC, [C, 1])
        c_ones1 = cload(ones1, [1, C])
        c_eps = const.tile([1, 1], F32)
        nc.vector.memset(c_eps, LN_EPS)
        if has_bias:
            c_bq = cload(bq, [C, 1])
            c_bk = cload(bk, [C, 1])
            c_bv = cload(bv, [C, 1])
            c_bo = cload(bo, [C, 1])
        if has_gb:
            c_gam = cload(gam, [C, 1])
            c_bet = cload(bet, [C, 1])

        for it in range(nt):
            sl = slice(it * T, (it + 1) * T)

            xt = xpool.tile([C, T], F32, tag="xt")
            xp_ = xpool.tile([C, T], F32, tag="xp")
            xn = xpool.tile([C, T], F32, tag="xn")
            nc.sync.dma_start(xt, x_t[:, sl])
            nc.sync.dma_start(xp_, x_p[:, sl])
            nc.sync.dma_start(xn, x_n[:, sl])
            xs_all = (xt, xp_, xn)

            # ---- projections + P = Q*K ----
            q_ps = ps1.tile([C, T], F32, tag="q")
            nc.tensor.matmul(q_ps, c_wqT, xt, start=True, stop=True)
            if Q_VIA_ACT_COPY or has_bias:
                q_sb = sb.tile([C, T], F32, tag="qs")
                if has_bias:
                    nc.scalar.activation(q_sb, q_ps, AF.Identity, bias=c_bq)
                else:
                    nc.scalar.activation(q_sb, q_ps, AF.Copy)
                q_op = q_sb
            else:
                q_op = q_ps

            p_cat = sb.tile([C, 3 * T], F32, tag="pcat")
            for s in range(3):
                k_ps = ps1.tile([C, T], F32, tag="k")
                nc.tensor.matmul(k_ps, c_wkT, xs_all[s], start=True, stop=True)
                if has_bias:
                    k_sb = sb.tile([C, T], F32, tag="ks")
                    nc.scalar.activation(k_sb, k_ps, AF.Identity, bias=c_bk)
                    k_op = k_sb
                else:
                    k_op = k_ps
                if ABL_DVE:
                    nc.sync.dma_start(p_cat[:, s * T:(s + 1) * T], q_op)
                else:
                    nc.vector.tensor_mul(
                        p_cat[:, s * T:(s + 1) * T], q_op, k_op
                    )

            # ---- scores -> softmax weights a (normalized) ----
            sc_ps = []
            for s in range(3):
                scp = ps1.tile([NH, T], F32, tag="sc")
                nc.tensor.matmul(
                    scp, c_maskC, p_cat[:, s * T:(s + 1) * T],
                    start=True, stop=True,
                )
                sc_ps.append(scp)
            e_cat = sb.tile([NH, 3 * T], F32, tag="ecat")
            for s in range(3):
                nc.scalar.activation(
                    e_cat[:, s * T:(s + 1) * T], sc_ps[s], AF.Exp
                )
            nrm1 = sb.tile([NH, T], F32, tag="nrm1")
            nrm2 = sb.tile([NH, T], F32, tag="nrm2")
            nc.gpsimd.tensor_add(nrm1, e_cat[:, 0:T], e_cat[:, T:2 * T])
            nc.gpsimd.tensor_add(nrm2, nrm1, e_cat[:, 2 * T:3 * T])
            r = sb.tile([NH, T], F32, tag="r")
            nc.vector.reciprocal(r, nrm2)
            # a = e * r (broadcast r across the 3 frames)
            a_cat = sb.tile([NH, 3 * T], F32, tag="acat")
            e_v = e_cat.rearrange("p (s t) -> p s t", s=3)
            a_v = a_cat.rearrange("p (s t) -> p s t", s=3)
            r_v = r.unsqueeze(1).broadcast_to([NH, 3, T])
            if ABL_DVE:
                nc.sync.dma_start(a_cat, e_cat)
            else:
                nc.vector.tensor_mul(a_v, e_v, r_v)

            # ---- ctx = sum_s A_s*V_s ; AO = Wo@ctx + x_t (PSUM accum) ----
            g_cat = sb.tile([C, 3 * T], F32, tag="gcat")
            v_sbs = []
            for s in range(3):
                v_ps = ps1.tile([C, T], F32, tag="v")
                nc.tensor.matmul(v_ps, c_wvT, xs_all[s], start=True, stop=True)
                v_sb = sb.tile([C, T], F32, tag="vs")
                if has_bias:
                    nc.scalar.activation(v_sb, v_ps, AF.Identity, bias=c_bv)
                else:
                    nc.scalar.activation(v_sb, v_ps, AF.Copy)
                v_sbs.append(v_sb)
            for s in range(3):
                ax_ps = ps1.tile([C, T], F32, tag="ax")
                nc.tensor.matmul(
                    ax_ps, c_maskT, a_cat[:, s * T:(s + 1) * T],
                    start=True, stop=True,
                )
                nc.vector.tensor_mul(
                    g_cat[:, s * T:(s + 1) * T], ax_ps, v_sbs[s]
                )
            # ctx = G0+G1+G2 on DVE (PE is the bottleneck engine)
            ctx1 = sb.tile([C, T], F32, tag="ctx1")
            nc.vector.tensor_add(ctx1, g_cat[:, 0:T], g_cat[:, T:2 * T])
            ctx2 = sb.tile([C, T], F32, tag="ctx2")
            nc.vector.tensor_add(ctx2, ctx1, g_cat[:, 2 * T:3 * T])

            ao_ps = ps1.tile([C, T], F32, tag="tail")
            nc.tensor.matmul(ao_ps, c_woT, ctx2, start=True, stop=True)

            # ---- LayerNorm over channels (partition axis) ----
            # y = attn_out + residual (DVE; also frees ao bank early)
            y = sb.tile([C, T], F32, tag="y")
            if has_bias:
                yb = sb.tile([C, T], F32, tag="yb")
                nc.scalar.activation(yb, ao_ps, AF.Identity, bias=c_bo)
                nc.vector.tensor_add(y, yb, xt)
            else:
                nc.vector.tensor_add(y, ao_ps, xt)
            y2 = sb.tile([C, T], F32, tag="y2")
            nc.scalar.activation(y2, y, AF.Square)

            s12_ps = ps1.tile([1, 2 * T], F32, tag="s12")
            s1_ps = s12_ps[:, 0:T]
            s2_ps = s12_ps[:, T:2 * T]
            nc.tensor.matmul(s1_ps, c_onesC, y, start=True, stop=True)
            nc.tensor.matmul(s2_ps, c_onesC, y2, start=True, stop=True)

            m2 = sb.tile([1, T], F32, tag="m2")
            nc.scalar.activation(m2, s1_ps, AF.Square, scale=1.0 / C)
            var = sb.tile([1, T], F32, tag="var")
            nc.vector.scalar_tensor_tensor(
                var, s2_ps, 1.0 / C, m2, ALU.mult, ALU.subtract
            )
            std = sb.tile([1, T], F32, tag="std")
            nc.scalar.activation(std, var, AF.Sqrt, bias=c_eps)
            rstd = sb.tile([1, T], F32, tag="rstd")
            nc.vector.reciprocal(rstd, std)
            qln = sb.tile([1, T], F32, tag="qln")
            nc.vector.scalar_tensor_tensor(
                qln, s1_ps, -1.0 / C, rstd, ALU.mult, ALU.mult
            )

            rx_ps = ps1.tile([C, T], F32, tag="tail")
            nc.tensor.matmul(rx_ps, c_ones1, rstd, start=True, stop=True)
            t1 = sb.tile([C, T], F32, tag="t1")
            nc.vector.tensor_mul(t1, y, rx_ps)
            qx_ps = ps1.tile([C, T], F32, tag="tail")
            nc.tensor.matmul(qx_ps, c_ones1, qln, start=True, stop=True)
            o_sb = sb.tile([C, T], F32, tag="osb")
            nc.vector.tensor_add(o_sb, t1, qx_ps)
            if has_gb:
                o2_sb = sb.tile([C, T], F32, tag="o2sb")
                nc.scalar.activation(
                    o2_sb, o_sb, AF.Identity, scale=c_gam, bias=c_bet
                )
                o_fin = o2_sb
            else:
                o_fin = o_sb

            nc.sync.dma_start(out[:, sl], o_fin)

    if SPLIT_WAITS:
        _split_sync_waits(nc)
    return nc


def _host_consts(in_proj_w, in_proj_b, out_w, out_b, gamma, beta):
    in_proj_w = np.asarray(in_proj_w, dtype=np.float32)
    in_proj_b = np.asarray(in_proj_b, dtype=np.float32)
    out_w = np.asarray(out_w, dtype=np.float32)
    out_b = np.asarray(out_b, dtype=np.float32)
    gamma = np.asarray(gamma, dtype=np.float32)
    beta = np.asarray(beta, dtype=np.float32)

    wq, wk, wv = in_proj_w[:C], in_proj_w[C:2 * C], in_proj_w[2 * C:]
    bq_, bk_, bv_ = in_proj_b[:C], in_proj_b[C:2 * C], in_proj_b[2 * C:]
    sc = 1.0 / math.sqrt(HD)

    consts = {
        "wqT": np.ascontiguousarray(wq.T) * sc,
        "wkT": np.ascontiguousarray(wk.T),
        "wvT": np.ascontiguousarray(wv.T),
        "woT": np.ascontiguousarray(out_w.T),
        "maskT": np.repeat(np.eye(NH, dtype=np.float32), HD, axis=1),
        "maskC": np.repeat(np.eye(NH, dtype=np.float32), HD, axis=0),
        "ident": np.eye(C, dtype=np.float32),
        "onesC": np.ones((C, 1), dtype=np.float32),
        "ones1": np.ones((1, C), dtype=np.float32),
    }
    has_bias = bool(
        np.any(in_proj_b != 0.0) or np.any(out_b != 0.0)
    )
    if has_bias:
        consts["bq"] = (bq_ * sc).reshape(C, 1)
        consts["bk"] = bk_.reshape(C, 1)
        consts["bv"] = bv_.reshape(C, 1)
        consts["bo"] = out_b.reshape(C, 1)
    has_gb = bool(np.any(gamma != 1.0) or np.any(beta != 0.0))
    if has_gb:
        consts["gam"] = gamma.reshape(C, 1)
        consts["bet"] = beta.reshape(C, 1)
    consts = {k: np.ascontiguousarray(v, dtype=np.float32) for k, v in consts.items()}
    return consts, has_bias, has_gb


def kernel(f_t, f_p, f_n, in_proj_w, in_proj_b, out_w, out_b, gamma, beta):
    f_t = np.asarray(f_t, dtype=np.float32)
    f_p = np.asarray(f_p, dtype=np.float32)
    f_n = np.asarray(f_n, dtype=np.float32)
    b, c, h, w = f_t.shape
    assert (b, c) == (B, C) and h * w == HW

    consts, has_bias, has_gb = _host_consts(
        in_proj_w, in_proj_b, out_w, out_b, gamma, beta
    )
    nc = build_nc(HW, has_bias, has_gb)

    in_maps = []
    for i in range(B):
        m = dict(consts)
        m["x_t"] = np.ascontiguousarray(f_t[i].reshape(C, HW))
        m["x_p"] = np.ascontiguousarray(f_p[i].reshape(C, HW))
        m["x_n"] = np.ascontiguousarray(f_n[i].reshape(C, HW))
        in_maps.append(m)

    res = run_bass_kernel_spmd(nc, in_maps, core_ids=list(range(N_CORES)))
    global LAST_RESULT
    LAST_RESULT = res
    outs = [res.results[i]["out"].reshape(C, h, w) for i in range(B)]
    return np.stack(outs, axis=0).astype(np.float32)


LAST_RESULT = None



# revision 5
# speedup vs baseline: 82.6915x; 82.6915x over previous
"""CrossFrameAttention Trainium2 kernel, v6 (walrus-legal, pipelined).

Math (per location n, C=128 ch, NH=4 heads, HD=32, frames s in {t,p,n}):
  q = Wq x_t; k_s = Wk x_s; v_s = Wv x_s
  sc_s[h] = <q_h, k_{s,h}>/sqrt(hd); a = softmax_s; ctx = sum_s a_s v_s
  y = Wo ctx + x_t; out = LayerNorm_C(y)

Engine/legality notes (walrus-verified):
  - fp32r matmuls at 1 PE cycle/col; producers (DMA) emit f32r dtype
  - Pool/GPSIMD is SBUF-only; DVE TensorTensor/STT take at most ONE PSUM
    operand; ACT reads PSUM; DMA is HBM<->SBUF / SBUF->SBUF only
  - scores in [68, T] block (frame s rows at partitions 32s..32s+3), one
    PSUM bank; softmax sum+broadcast via single m68 matmul (identity rows
    keep unused partitions at exp(0)=1); r68 = DVE reciprocal
  - softmax/value path bf16: P, E, r68, A, AXc, G, y, y2
  - one ACT table: Exp/Square/Identity/Copy/Ln; rstd = exp(-.5 ln(var+eps))
  - LN rstd/-mu*rstd broadcast via DMA (stride-0 free-dim source)
  - 8-stage software pipeline; PSUM tags qAO/KAX/V/hel/S = 8 banks
"""

import math
import sys
from contextlib import ExitStack

import numpy as np

sys.path.insert(0, "/opt/trn_rl_repo")

import concourse.bass as bass
import concourse.tile as tile
from concourse import mybir
from concourse.bass_utils import run_bass_kernel_spmd

C = 128
NH = 4
HD = C // NH
B = 8
HW = 128 * 128
T = 512
LN_EPS = 1e-5
SB = 68

F32 = mybir.dt.float32
F32R = mybir.dt.float32r
BF16 = mybir.dt.bfloat16
AF = mybir.ActivationFunctionType
ALU = mybir.AluOpType

SPLIT_WAITS = True
N_CORES = 8


def _split_sync_waits(nc: bass.Bass, max_waits: int = 1) -> None:
    f = nc.m.functions[0]
    for bb in f.blocks:
        new = []
        for inst in bb.instructions:
            si = getattr(inst, "sync_info", None)
            limit = 1 if isinstance(inst, mybir.InstDrain) else max_waits
            if si is not None and si.on_wait and len(si.on_wait) > limit:
                waits = list(si.on_wait)
                extra, keep = waits[:-limit], waits[-limit:]
                for k in range(0, len(extra), 1):
                    new.append(mybir.InstDrain(
                        name=f"{inst.name}-sw{k}",
                        engine=inst.engine,
                        ins=[], outs=[],
                        sync_info=mybir.SyncInfo(
                            on_wait=[extra[k]], on_update=[]),
                    ))
                inst.sync_info = mybir.SyncInfo(
                    on_wait=keep, on_update=list(si.on_update))
            new.append(inst)
        bb.instructions[:] = new


def build_nc(ncols: int, has_bias: bool, has_gb: bool) -> bass.Bass:
    nt = ncols // T
    assert ncols % T == 0
    T3 = 3 * T

    nc = bass.Bass("TRN2")

    x_t = nc.dram_tensor("x_t", [C, ncols], F32, kind="ExternalInput")
    x_p = nc.dram_tensor("x_p", [C, ncols], F32, kind="ExternalInput")
    x_n = nc.dram_tensor("x_n", [C, ncols], F32, kind="ExternalInput")
    out = nc.dram_tensor("out", [C, ncols], F32, kind="ExternalOutput")

    wqT = nc.dram_tensor("wqT", [C, C], F32, kind="ExternalInput")
    wkT = nc.dram_tensor("wkT", [C, C], F32, kind="ExternalInput")
    wvT = nc.dram_tensor("wvT", [C, C], F32, kind="ExternalInput")
    woTb = nc.dram_tensor("woTb", [C, C], BF16, kind="ExternalInput")
    maskCb = nc.dram_tensor("maskCb", [C, 3 * SB], BF16,
                            kind="ExternalInput")
    m68 = nc.dram_tensor("m68", [SB, SB], BF16, kind="ExternalInput")
    mT3 = nc.dram_tensor("mT3", [SB, C], BF16, kind="ExternalInput")
    onesCb = nc.dram_tensor("onesCb", [C, 1], BF16, kind="ExternalInput")
    if has_bias:
        bq = nc.dram_tensor("bq", [C, 1], F32, kind="ExternalInput")
        bk = nc.dram_tensor("bk", [C, 1], F32, kind="ExternalInput")
        bv = nc.dram_tensor("bv", [C, 1], F32, kind="ExternalInput")
        bo = nc.dram_tensor("bo", [C, 1], F32, kind="ExternalInput")
    if has_gb:
        gam = nc.dram_tensor("gam", [C, 1], F32, kind="ExternalInput")
        bet = nc.dram_tensor("bet", [C, 1], F32, kind="ExternalInput")

    with tile.TileContext(nc) as tc, ExitStack() as ctx:
        const = ctx.enter_context(tc.tile_pool(name="const", bufs=1))
        xpool = ctx.enter_context(tc.tile_pool(name="xpool", bufs=7))
        sb = ctx.enter_context(tc.tile_pool(name="sb", bufs=2))
        ps = ctx.enter_context(tc.tile_pool(name="ps", bufs=1, space="PSUM"))

        def cload(dr, shape, dt=F32):
            t = const.tile(shape, dt, tag=f"c_{dr.name}")
            if dt == F32R:
                nc.sync.dma_start(t, dr[:, :].bitcast(F32R))
            else:
                nc.sync.dma_start(t, dr[:, :])
            return t

        c_wqT = cload(wqT, [C, C], F32R)
        c_wkT = cload(wkT, [C, C], F32R)
        c_wvT = cload(wvT, [C, C], F32R)
        c_woTb = cload(woTb, [C, C], BF16)
        c_maskCb = cload(maskCb, [C, 3 * SB], BF16)
        c_m68 = cload(m68, [SB, SB], BF16)
        c_mT3 = cload(mT3, [SB, C], BF16)
        c_onesCb = cload(onesCb, [C, 1], BF16)
        c_eps = const.tile([1, 1], F32)
        nc.vector.memset(c_eps, LN_EPS)
        if has_bias:
            c_bq = cload(bq, [C, 1])
            c_bk = cload(bk, [C, 1])
            c_bv = cload(bv, [C, 1])
            c_bo = cload(bo, [C, 1])
        if has_gb:
            c_gam = cload(gam, [C, 1])
            c_bet = cload(bet, [C, 1])

        state = {}

        # ---------------- pipeline stages ----------------
        def stage_L(j):
            sl = slice(j * T, (j + 1) * T)
            X = xpool.tile([C, T3], F32R, tag="X", bufs=7)
            nc.sync.dma_start(X[:, 0:T], x_t[:, sl].bitcast(F32R))
            nc.sync.dma_start(X[:, T:2 * T], x_p[:, sl].bitcast(F32R))
            nc.sync.dma_start(X[:, 2 * T:3 * T], x_n[:, sl].bitcast(F32R))
            state[("X", j)] = X

        def stage_F(j):
            X = state[("X", j)]
            q_ps = ps.tile([C, T], F32, tag="qAO", bufs=1)
            nc.tensor.matmul(q_ps, c_wqT, X[:, 0:T], start=True, stop=True)
            q_sb = sb.tile([C, T], BF16, tag="qs")
            if has_bias:
                nc.scalar.activation(q_sb, q_ps, AF.Identity, bias=c_bq)
            else:
                nc.scalar.activation(q_sb, q_ps, AF.Copy)
            P = sb.tile([C, T3], BF16, tag="P")
            for s in range(3):
                K_ps = ps.tile([C, T], F32, tag="KAX", bufs=2)
                nc.tensor.matmul(K_ps, c_wkT, X[:, s * T:(s + 1) * T],
                                 start=True, stop=True)
                if has_bias:
                    nc.vector.scalar_tensor_tensor(
                        P[:, s * T:(s + 1) * T], K_ps, c_bk, q_sb,
                        ALU.add, ALU.mult)
                else:
                    nc.vector.tensor_mul(
                        P[:, s * T:(s + 1) * T], K_ps, q_sb)
            state[("P", j)] = P

        def stage_M1(j):
            P = state.pop(("P", j))
            SC_ps = ps.tile([SB, T], F32, tag="hel", bufs=1)
            for s in range(3):
                nc.tensor.matmul(SC_ps, c_maskCb[:, s * SB:(s + 1) * SB],
                                 P[:, s * T:(s + 1) * T],
                                 start=(s == 0), stop=(s == 2))
            E = sb.tile([SB, T], BF16, tag="E")
            nc.scalar.activation(E, SC_ps, AF.Exp)
            state[("E", j)] = E

        def stage_M2(j):
            E = state.pop(("E", j))
            nrm_ps = ps.tile([SB, T], F32, tag="hel", bufs=1)
            nc.tensor.matmul(nrm_ps, c_m68, E, start=True, stop=True)
            r68 = sb.tile([SB, T], BF16, tag="r68")
            with nc.allow_low_precision("3-way softmax weights; 2e-2 tol"):
                nc.vector.reciprocal(r68, nrm_ps)
            A = sb.tile([SB, T], BF16, tag="A")
            nc.gpsimd.tensor_mul(A, E, r68)
            state[("A", j)] = A

        def stage_B1a(j):
            X = state[("X", j)]
            A = state.pop(("A", j))
            AXc = sb.tile([C, T3], BF16, tag="AXc")
            for s in range(3):
                AX_ps = ps.tile([C, T], F32, tag="KAX", bufs=2)
                nc.tensor.matmul(AX_ps, c_mT3[32 * s:32 * s + NH, :],
                                 A[32 * s:32 * s + NH, :],
                                 start=True, stop=True)
                nc.scalar.activation(AXc[:, s * T:(s + 1) * T], AX_ps,
                                     AF.Copy)
                V_ps = ps.tile([C, T], F32, tag="V", bufs=2)
                nc.tensor.matmul(V_ps, c_wvT, X[:, s * T:(s + 1) * T],
                                 start=True, stop=True)
                state[("V", j, s)] = V_ps
            state[("AXc", j)] = AXc

        def stage_B1b(j):
            X = state[("X", j)]
            AXc = state.pop(("AXc", j))
            xt = X[:, 0:T]
            G = sb.tile([C, T3], BF16, tag="G")
            for s in range(3):
                V_ps = state.pop(("V", j, s))
                if has_bias:
                    Vb = sb.tile([C, T], F32, tag="Vb")
                    nc.scalar.activation(Vb, V_ps, AF.Identity, bias=c_bv)
                    nc.vector.tensor_mul(
                        G[:, s * T:(s + 1) * T], Vb,
                        AXc[:, s * T:(s + 1) * T])
                else:
                    nc.vector.tensor_mul(
                        G[:, s * T:(s + 1) * T], V_ps,
                        AXc[:, s * T:(s + 1) * T])
            AO_ps = ps.tile([C, T], F32, tag="qAO", bufs=1)
            for s in range(3):
                nc.tensor.matmul(AO_ps, c_woTb, G[:, s * T:(s + 1) * T],
                                 start=(s == 0), stop=(s == 2))
            y = sb.tile([C, T], BF16, tag="y", bufs=3)
            if has_bias:
                nc.vector.scalar_tensor_tensor(
                    y, AO_ps, c_bo, xt.bitcast(F32), ALU.add, ALU.add)
            else:
                nc.vector.tensor_add(y, AO_ps, xt.bitcast(F32))
            y2 = sb.tile([C, T], BF16, tag="y2")
            nc.gpsimd.tensor_mul(y2, y, y)
            mu_ps = ps.tile([1, T], F32, tag="S", bufs=2)
            nc.tensor.matmul(mu_ps, c_onesCb, y, start=True, stop=True)
            msq_ps = ps.tile([1, T], F32, tag="S", bufs=2)
            nc.tensor.matmul(msq_ps, c_onesCb, y2, start=True, stop=True)
            state[("y", j)] = y
            state[("mu", j)] = mu_ps
            state[("msq", j)] = msq_ps

        def stage_C1(j):
            mu_ps = state.pop(("mu", j))
            msq_ps = state.pop(("msq", j))
            muc = sb.tile([1, T], F32, tag="muc")
            nc.scalar.activation(muc, mu_ps, AF.Copy)
            msqc = sb.tile([1, T], F32, tag="msqc")
            nc.scalar.activation(msqc, msq_ps, AF.Copy)
            m2 = sb.tile([1, T], F32, tag="m2")
            nc.gpsimd.tensor_mul(m2, muc, muc)
            var = sb.tile([1, T], F32, tag="var")
            nc.gpsimd.tensor_sub(var, msqc, m2)
            lnv = sb.tile([1, T], F32, tag="lnv")
            nc.scalar.activation(lnv, var, AF.Ln, bias=c_eps)
            rstd = sb.tile([1, T], F32, tag="rstd")
            nc.scalar.activation(rstd, lnv, AF.Exp, scale=-0.5)
            qln = sb.tile([1, T], F32, tag="qln")
            nc.vector.scalar_tensor_tensor(
                qln, muc, -1.0, rstd, ALU.mult, ALU.mult)
            state[("rstd", j)] = rstd
            state[("qln", j)] = qln

        def stage_C2(j):
            sl = slice(j * T, (j + 1) * T)
            y = state.pop(("y", j))
            rstd = state.pop(("rstd", j))
            qln = state.pop(("qln", j))
            del state[("X", j)]
            RX = sb.tile([C, T], F32, tag="RX")
            nc.sync.dma_start(RX, rstd.unsqueeze(1).broadcast_to([1, C, T]))
            QX = sb.tile([C, T], F32, tag="QX")
            nc.sync.dma_start(QX, qln.unsqueeze(1).broadcast_to([1, C, T]))
            t1 = sb.tile([C, T], F32, tag="t1")
            nc.gpsimd.tensor_mul(t1, y, RX)
            o_sb = sb.tile([C, T], F32, tag="osb")
            if has_gb:
                o0 = sb.tile([C, T], F32, tag="o0")
                nc.gpsimd.tensor_add(o0, t1, QX)
                nc.scalar.activation(o_sb, o0, AF.Identity,
                                     scale=c_gam, bias=c_bet)
            else:
                nc.gpsimd.tensor_add(o_sb, t1, QX)
            nc.sync.dma_start(out[:, sl], o_sb)

        stages = [stage_L, stage_F, stage_M1, stage_M2,
                  stage_B1a, stage_B1b, stage_C1, stage_C2]
        NS = len(stages)
        for i in range(nt + NS - 1):
            if i < nt:
                stage_L(i)
            for k in range(NS - 1, 0, -1):
                j = i - k
                if 0 <= j < nt:
                    stages[k](j)

    if SPLIT_WAITS:
        _split_sync_waits(nc)
    return nc


def _host_consts(in_proj_w, in_proj_b, out_w, out_b, gamma, beta):
    import ml_dtypes
    in_proj_w = np.asarray(in_proj_w, dtype=np.float32)
    in_proj_b = np.asarray(in_proj_b, dtype=np.float32)
    out_w = np.asarray(out_w, dtype=np.float32)
    out_b = np.asarray(out_b, dtype=np.float32)
    gamma = np.asarray(gamma, dtype=np.float32)
    beta = np.asarray(beta, dtype=np.float32)

    wq, wk, wv = in_proj_w[:C], in_proj_w[C:2 * C], in_proj_w[2 * C:]
    bq_, bk_, bv_ = in_proj_b[:C], in_proj_b[C:2 * C], in_proj_b[2 * C:]
    sc = 1.0 / math.sqrt(HD)

    maskC = np.repeat(np.eye(NH, dtype=np.float32), HD, axis=0)
    maskC68 = np.zeros((C, 3 * SB), np.float32)
    m68 = np.eye(SB, dtype=np.float32)
    for s in range(3):
        for h in range(NH):
            maskC68[:, s * SB + 32 * s + h] = maskC[:, h]
            m68[32 * s + h, 32 * s + h] = 0.0
    for h in range(NH):
        for s in range(3):
            for sp in range(3):
                m68[32 * sp + h, 32 * s + h] = 1.0
    maskT = np.repeat(np.eye(NH, dtype=np.float32), HD, axis=1)
    mT3 = np.zeros((SB, C), np.float32)
    for s in range(3):
        for h in range(NH):
            mT3[32 * s + h] = maskT[h]

    bf = ml_dtypes.bfloat16
    consts = {
        "wqT": (np.ascontiguousarray(wq.T) * sc).astype(np.float32),
        "wkT": np.ascontiguousarray(wk.T).astype(np.float32),
        "wvT": np.ascontiguousarray(wv.T).astype(np.float32),
        "woTb": np.ascontiguousarray(out_w.T).astype(bf),
        "maskCb": maskC68.astype(bf),
        "m68": m68.astype(bf),
        "mT3": mT3.astype(bf),
        "onesCb": np.full((C, 1), 1.0 / C, dtype=np.float32).astype(bf),
    }
    has_bias = bool(np.any(in_proj_b != 0.0) or np.any(out_b != 0.0))
    if has_bias:
        consts["bq"] = (bq_ * sc).reshape(C, 1).astype(np.float32)
        consts["bk"] = bk_.reshape(C, 1).astype(np.float32)
        consts["bv"] = bv_.reshape(C, 1).astype(np.float32)
        consts["bo"] = out_b.reshape(C, 1).astype(np.float32)
    has_gb = bool(np.any(gamma != 1.0) or np.any(beta != 0.0))
    if has_gb:
        consts["gam"] = gamma.reshape(C, 1).astype(np.float32)
        consts["bet"] = beta.reshape(C, 1).astype(np.float32)
    return consts, has_bias, has_gb


def kernel(f_t, f_p, f_n, in_proj_w, in_proj_b, out_w, out_b, gamma, beta):
    f_t = np.asarray(f_t, dtype=np.float32)
    f_p = np.asarray(f_p, dtype=np.float32)
    f_n = np.asarray(f_n, dtype=np.float32)
    b, c, h, w = f_t.shape
    assert (b, c) == (B, C) and h * w == HW

    consts, has_bias, has_gb = _host_consts(
        in_proj_w, in_proj_b, out_w, out_b, gamma, beta
    )
    nc = build_nc(HW, has_bias, has_gb)

    in_maps = []
    for i in range(B):
        m = dict(consts)
        m["x_t"] = np.ascontiguousarray(f_t[i].reshape(C, HW))
        m["x_p"] = np.ascontiguousarray(f_p[i].reshape(C, HW))
        m["x_n"] = np.ascontiguousarray(f_n[i].reshape(C, HW))
        in_maps.append(m)

    res = run_bass_kernel_spmd(nc, in_maps, core_ids=list(range(N_CORES)))
    global LAST_RESULT
    LAST_RESULT = res
    outs = [res.results[i]["out"].reshape(C, h, w) for i in range(B)]
    return np.stack(outs, axis=0).astype(np.float32)


LAST_RESULT = None
